# revision 29
# baseline (speedup 1.0000x reference)
"""Fused multi-head attention block (QKV -> softmax attention -> proj) on 8
TRN2 NeuronCores.

Sharding: data-parallel over batch (2) x tensor-parallel over heads (12 heads
-> 4 groups of 3). Core c handles batch c//4, heads 3*(c%4)..3*(c%4)+2.
Each core computes a rank-192 partial of the output projection; the host sums
the 4 partials per batch and adds proj bias.

Per-core layout (feature-major, fp16):
  q01/k01: [h0 d(0:64) ; h1 d(64:128)] x tokens -- K=64 score matmuls for the
  two heads run concurrently via PE row-tiling (partitions 0-63 / 64-127).
  q22/k22: head2 duplicated on both partition halves so even/odd j-tiles pair.
  v is PE-transposed to token-major with a ones column appended (v_aug), so
  the PV matmul also produces the softmax denominator on partition 64.
"""

import numpy as np

import concourse.bass as bass
import concourse.mybir as mybir
import concourse.tile as tile
from concourse import bacc
from concourse.bass_utils import run_bass_kernel_spmd
from concourse.masks import make_identity

F16 = mybir.dt.float16
F32 = mybir.dt.float32
EXP = mybir.ActivationFunctionType.Exp

B = 2            # batch
N = 4096         # tokens (64*64)
C = 768          # channels
NH = 12          # heads
HD = 64          # head dim
HPC = 3          # heads per core
NCORES = 8
SCALE = HD ** -0.5

NT = N // 128    # 32 j-tiles
NIB = N // 512   # 8 i-blocks
NTB = N // 512   # 8 token blocks (phase 1)
NKT = C // 128   # 6 contraction tiles
NF = 704         # features computed in phase 1 (q01,k01,v01,q22,k22,v2)


def _build():
    nc = bacc.Bacc("TRN2", target_bir_lowering=False, debug=False,
                   num_devices=NCORES)

    xT = nc.dram_tensor("xT", [C, N], F16, kind="ExternalInput").ap()
    w = nc.dram_tensor("w", [C, NF], F16, kind="ExternalInput").ap()
    bias = nc.dram_tensor("bias", [128, 6], F32, kind="ExternalInput").ap()
    pw = nc.dram_tensor("pw", [HPC * HD, C], F16, kind="ExternalInput").ap()
    y = nc.dram_tensor("y", [C, N], F16, kind="ExternalOutput").ap()

    xT_r = xT.rearrange("(kt p) (tb n) -> p tb kt n", p=128, n=512)
    w_r = w.rearrange("(kt p) f -> p kt f", p=128)

    with tile.TileContext(nc) as tc:
        with (
            tc.tile_pool(name="singles", bufs=1) as singles,
            tc.tile_pool(name="bigs", bufs=1) as bigs,
            tc.tile_pool(name="xin", bufs=3) as xin,
            tc.tile_pool(name="exp", bufs=8) as expool,
            tc.tile_pool(name="outs", bufs=2) as outs,
            tc.tile_pool(name="psum", bufs=2, space="PSUM") as psum,
            tc.tile_pool(name="dram", bufs=1, space="DRAM") as dram,
        ):
            # ---- constants / weights ----
            w_sb = singles.tile([128, NKT, NF], F16)
            nc.sync.dma_start(out=w_sb, in_=w_r)
            bias_sb = singles.tile([128, 6], F32)
            nc.sync.dma_start(out=bias_sb, in_=bias)
            pwa = singles.tile([128, C], F16)
            nc.sync.dma_start(out=pwa, in_=pw[0:128, :])
            pwb = singles.tile([64, C], F16)
            nc.sync.dma_start(out=pwb, in_=pw[128:192, :])
            ident = singles.tile([128, 128], F16)
            make_identity(nc, ident)

            # ---- phase-1 destinations (feature-major, fp16) ----
            q01 = bigs.tile([128, N], F16)
            k01 = bigs.tile([128, N], F16)
            v01T = bigs.tile([128, N], F16)
            q22 = bigs.tile([128, N], F16)
            k22 = bigs.tile([128, N], F16)
            v2T = bigs.tile([64, N], F16)
            dests = [q01, k01, v01T, q22, k22, v2T]
            msizes = [128, 128, 128, 128, 128, 64]
            moffs = [0, 128, 256, 384, 512, 640]
            vaug = [bigs.tile([128, NT, 65], F16, name=f"vaug{h}",
                              tag=f"vaug{h}")
                    for h in range(HPC)]
            for h in range(HPC):
                nc.vector.memset(vaug[h][:, :, 64:65], 1.0)

            # ---- phase 1: qkv projection (k/q first so attention overlaps;
            #      two m-tiles per x load) ----
            for mta, mtb in ((1, 0), (2, 4), (3, 5)):
                for tb in range(NTB):
                    x_t = xin.tile([128, NKT, 512], F16, bufs=3)
                    nc.sync.dma_start(out=x_t, in_=xT_r[:, tb, :, :])
                    for mt in (mta, mtb):
                        msz = msizes[mt]
                        ps = psum.tile([128, 512], F32, tag="ps512", bufs=2)
                        for kt in range(NKT):
                            nc.tensor.matmul(
                                ps[0:msz, :],
                                lhsT=w_sb[:, kt, moffs[mt]:moffs[mt] + msz],
                                rhs=x_t[:, kt, :],
                                start=(kt == 0), stop=(kt == NKT - 1),
                            )
                        nc.vector.tensor_scalar_add(
                            out=dests[mt][0:msz, tb * 512:(tb + 1) * 512],
                            in0=ps[0:msz, :],
                            scalar1=bias_sb[0:msz, mt:mt + 1],
                        )
                    if mta == 2:
                        # v01 slice ready: transpose its 4 j-tiles now
                        for jt in range(4 * tb, 4 * tb + 4):
                            tp = psum.tile([128, 128], F16, tag="ps512",
                                           bufs=2)
                            nc.tensor.transpose(
                                tp, v01T[:, jt * 128:(jt + 1) * 128], ident)
                            nc.vector.tensor_copy(out=vaug[0][:, jt, 0:64],
                                                  in_=tp[:, 0:64])
                            nc.vector.tensor_copy(out=vaug[1][:, jt, 0:64],
                                                  in_=tp[:, 64:128])
                    if mtb == 5:
                        for jt in range(4 * tb, 4 * tb + 4):
                            tp2 = psum.tile([128, 64], F16, tag="ps512",
                                            bufs=2)
                            nc.tensor.transpose(
                                tp2, v2T[:, jt * 128:(jt + 1) * 128],
                                ident[0:64, 0:64])
                            nc.vector.tensor_copy(out=vaug[2][:, jt, 0:64],
                                                  in_=tp2)

            # ---- attention ----
            o01 = bigs.tile([128, N], F16)
            o2 = bigs.tile([64, N], F16)
            rec_d = dram.tile([24, 512], F32)     # 1/sums bounce for p-bcast


            def emit_proj(ib, tags):
                isl = slice(ib * 512, (ib + 1) * 512)
                for mtp in range(0, 6, 2):
                    psys = []
                    for k in range(2):
                        mt = mtp + k
                        msl = slice(mt * 128, (mt + 1) * 128)
                        psy = psum.tile([128, 512], F32,
                                        tag=tags[k % len(tags)],
                                        bufs=2, name=f"psy{k}")
                        nc.tensor.matmul(psy, lhsT=pwa[:, msl],
                                         rhs=o01[:, isl],
                                         start=True, stop=False)
                        psys.append((psy, mt))
                    for psy, mt in psys:
                        msl = slice(mt * 128, (mt + 1) * 128)
                        nc.tensor.matmul(psy, lhsT=pwb[:, msl],
                                         rhs=o2[:, isl],
                                         start=False, stop=True)
                        ysb = outs.tile([128, 512], F16, tag="ysb", bufs=4)
                        nc.vector.tensor_copy(out=ysb, in_=psy)
                        nc.sync.dma_start(out=y[msl, isl], in_=ysb)

            def emit_pair_scores(ib, jt):
                isl2 = slice(ib * 512, (ib + 1) * 512)
                jsl = slice(jt * 128, (jt + 1) * 128)
                sc = psum.tile([128, 1024], F32, tag="sc", bufs=2, name="sc")
                nc.tensor.matmul(sc[:, 0:512], lhsT=k01[0:64, jsl],
                                 rhs=q01[0:64, isl2], start=True, stop=True)
                nc.tensor.matmul(sc[:, 512:1024], lhsT=k01[64:128, jsl],
                                 rhs=q01[64:128, isl2], start=True, stop=True)
                ex = expool.tile([128, 1024], F16, name="ex")
                nc.scalar.activation(ex, sc, EXP)
                return ex

            def emit_norm(ib, coll):
                isl = slice(ib * 512, (ib + 1) * 512)
                rec = outs.tile([3, 512], F32, tag="rec")
                nc.vector.reciprocal(out=rec, in_=coll)
                nc.sync.dma_start(out=rec_d[3 * ib:3 * ib + 3, :], in_=rec)
                for h in range(HPC):
                    rb = outs.tile([128, 512], F32, tag="rb")
                    src = rec_d[3 * ib + h:3 * ib + h + 1, :]
                    if h == 1:
                        rbv = rb[64:128, :]
                        dst = o01[64:128, isl]
                    elif h == 0:
                        rbv = rb[0:64, :]
                        dst = o01[0:64, isl]
                    else:
                        rbv = rb[0:64, :]
                        dst = o2[:, isl]
                    nc.sync.dma_start(out=rbv, in_=src.broadcast_to([64, 512]))
                    nc.vector.tensor_mul(dst, dst, rbv)

            prev = None
            preload = []
            for ib in range(NIB):
                isl = slice(ib * 512, (ib + 1) * 512)
                coll = outs.tile([3, 512], F32, tag="coll")

                # -- head pair (h0, h1): concurrent scores via row halves --
                pv0 = psum.tile([65, 512], F32, tag="pv", bufs=2)
                pv1 = psum.tile([65, 512], F32, tag="pv", bufs=2)
                for jt in range(NT):
                    if jt < len(preload):
                        ex = preload[jt]
                    else:
                        ex = emit_pair_scores(ib, jt)
                    nc.tensor.matmul(pv0, lhsT=vaug[0][:, jt, :],
                                     rhs=ex[:, 0:512],
                                     start=(jt == 0), stop=(jt == NT - 1))
                    nc.tensor.matmul(pv1, lhsT=vaug[1][:, jt, :],
                                     rhs=ex[:, 512:1024],
                                     start=(jt == 0), stop=(jt == NT - 1))
                nc.vector.tensor_copy(out=o01[0:64, isl], in_=pv0[0:64, :])
                nc.vector.tensor_copy(out=o01[64:128, isl], in_=pv1[0:64, :])
                sst0 = outs.tile([128, 512], F32, tag="sstage")
                nc.vector.tensor_copy(out=sst0[64:65, :], in_=pv0[64:65, :])
                nc.sync.dma_start(out=coll[0:1, :], in_=sst0[64:65, :])
                sst1 = outs.tile([128, 512], F32, tag="sstage")
                nc.vector.tensor_copy(out=sst1[64:65, :], in_=pv1[64:65, :])
                nc.sync.dma_start(out=coll[1:2, :], in_=sst1[64:65, :])
                preload = []
                if ib + 1 < NIB:
                    preload = [emit_pair_scores(ib + 1, jtp)
                               for jtp in range(4)]

                # -- head 2: even/odd j-tiles concurrent via row halves --
                pv2 = psum.tile([65, 512], F32, tag="pv", bufs=2)
                for jg in range(NT // 2):
                    jte, jto = 2 * jg, 2 * jg + 1
                    esl = slice(jte * 128, (jte + 1) * 128)
                    osl = slice(jto * 128, (jto + 1) * 128)
                    sc2 = psum.tile([128, 1024], F32, tag="sc", bufs=2)
                    nc.tensor.matmul(sc2[:, 0:512], lhsT=k22[0:64, esl],
                                     rhs=q22[0:64, isl], start=True, stop=True)
                    nc.tensor.matmul(sc2[:, 512:1024], lhsT=k22[64:128, osl],
                                     rhs=q22[64:128, isl], start=True, stop=True)
                    ex2 = expool.tile([128, 1024], F16)
                    nc.scalar.activation(ex2, sc2, EXP)
                    nc.tensor.matmul(pv2, lhsT=vaug[2][:, jte, :],
                                     rhs=ex2[:, 0:512],
                                     start=(jg == 0), stop=False)
                    nc.tensor.matmul(pv2, lhsT=vaug[2][:, jto, :],
                                     rhs=ex2[:, 512:1024],
                                     start=False, stop=(jg == NT // 2 - 1))
                nc.vector.tensor_copy(out=o2[:, isl], in_=pv2[0:64, :])
                sst2 = outs.tile([128, 512], F32, tag="sstage")
                nc.vector.tensor_copy(out=sst2[64:65, :], in_=pv2[64:65, :])
                nc.sync.dma_start(out=coll[2:3, :], in_=sst2[64:65, :])

                # -- normalize the previous i-block (stagger smooths FIFO) --
                if prev is not None:
                    emit_norm(prev[0], prev[1])
                prev = (ib, coll)
            emit_norm(prev[0], prev[1])

            for ib in range(NIB):
                emit_proj(ib, ("ps512", "pv", "sc"))

    nc.finalize()
    return nc


_NC_CACHE = None


def _get_nc():
    global _NC_CACHE
    if _NC_CACHE is None:
        _NC_CACHE = _build()
    return _NC_CACHE


def _prep_core_inputs(x, qkv_w, qkv_b, proj_w, core):
    """Build the per-core input dict (numpy, host-side)."""
    b, g = core // 4, core % 4
    h = [3 * g, 3 * g + 1, 3 * g + 2]

    xT = np.ascontiguousarray(
        x[b].reshape(N, C).T.astype(np.float16))          # (768, 4096)

    def wq(head):  # scaled q rows, (64, 768)
        return qkv_w[HD * head:HD * (head + 1), :] * SCALE

    def wk(head):
        return qkv_w[C + HD * head:C + HD * (head + 1), :]

    def wv(head):
        return qkv_w[2 * C + HD * head:2 * C + HD * (head + 1), :]

    def bq(head):
        return qkv_b[HD * head:HD * (head + 1)] * SCALE

    def bk(head):
        return qkv_b[C + HD * head:C + HD * (head + 1)]

    def bv(head):
        return qkv_b[2 * C + HD * head:2 * C + HD * (head + 1)]

    # feature columns: q01 | k01 | v01 | q22 | k22 | v2   (704 total)
    wcols = np.concatenate([
        wq(h[0]), wq(h[1]), wk(h[0]), wk(h[1]), wv(h[0]), wv(h[1]),
        wq(h[2]), wq(h[2]), wk(h[2]), wk(h[2]), wv(h[2]),
    ], axis=0)                                            # (704, 768)
    w = np.ascontiguousarray(wcols.T.astype(np.float16))  # (768, 704)

    bcols = np.concatenate([
        bq(h[0]), bq(h[1]), bk(h[0]), bk(h[1]), bv(h[0]), bv(h[1]),
        bq(h[2]), bq(h[2]), bk(h[2]), bk(h[2]), bv(h[2]),
        np.zeros(64, np.float32),
    ]).astype(np.float32)                                 # (768,)
    bias = np.ascontiguousarray(bcols.reshape(6, 128).T)  # (128, 6)

    ch = slice(HPC * HD * g, HPC * HD * (g + 1))
    pw = np.ascontiguousarray(proj_w[:, ch].T.astype(np.float16))  # (192, 768)

    return {"xT": xT, "w": w, "bias": bias, "pw": pw}


def kernel(x, qkv_w, qkv_b, proj_w, proj_b):
    x = np.asarray(x, np.float32)
    qkv_w = np.asarray(qkv_w, np.float32)
    qkv_b = np.asarray(qkv_b, np.float32)
    proj_w = np.asarray(proj_w, np.float32)
    proj_b = np.asarray(proj_b, np.float32)

    nc = _get_nc()
    in_maps = [_prep_core_inputs(x, qkv_w, qkv_b, proj_w, c)
               for c in range(NCORES)]
    res = run_bass_kernel_spmd(nc, in_maps, list(range(NCORES)))

    out = np.empty((B, N, C), np.float32)
    for b in range(B):
        acc = np.zeros((C, N), np.float32)
        for g in range(4):
            acc += res.results[b * 4 + g]["y"].astype(np.float32)
        out[b] = acc.T + proj_b[None, :]
    return out


if __name__ == "__main__":
    rng = np.random.default_rng(0)
    x = rng.standard_normal((B, 64, 64, C), np.float32)
    qkv_w = (rng.standard_normal((3 * C, C), np.float32) * 0.02)
    qkv_b = (rng.standard_normal(3 * C, np.float32) * 0.02)
    proj_w = (rng.standard_normal((C, C), np.float32) * 0.02)
    proj_b = (rng.standard_normal(C, np.float32) * 0.02)
    out = kernel(x=x, qkv_w=qkv_w, qkv_b=qkv_b, proj_w=proj_w, proj_b=proj_b)
    print("out", out.shape, out.dtype, float(np.abs(out).max()))


# revision 30
# speedup vs baseline: 1.0213x; 1.0213x over previous
"""Fused multi-head attention block (QKV -> softmax attention -> proj) on 8
TRN2 NeuronCores.

Sharding: data-parallel over batch (2) x tensor-parallel over heads (12 heads
-> 4 groups of 3). Core c handles batch c//4, heads 3*(c%4)..3*(c%4)+2.
Each core computes a rank-192 partial of the output projection; the host sums
the 4 partials per batch and adds proj bias.

Per-core layout (feature-major, fp16):
  q01/k01: [h0 d(0:64) ; h1 d(64:128)] x tokens -- K=64 score matmuls for the
  two heads run concurrently via PE row-tiling (partitions 0-63 / 64-127).
  q22/k22: head2 duplicated on both partition halves so even/odd j-tiles pair.
  v is PE-transposed to token-major with a ones column appended (v_aug), so
  the PV matmul also produces the softmax denominator on partition 64.
"""

import numpy as np

import concourse.bass as bass
import concourse.mybir as mybir
import concourse.tile as tile
from concourse import bacc
from concourse.bass_utils import run_bass_kernel_spmd
from concourse.masks import make_identity

F16 = mybir.dt.float16
F32 = mybir.dt.float32
EXP = mybir.ActivationFunctionType.Exp

B = 2            # batch
N = 4096         # tokens (64*64)
C = 768          # channels
NH = 12          # heads
HD = 64          # head dim
HPC = 3          # heads per core
NCORES = 8
SCALE = HD ** -0.5

NT = N // 128    # 32 j-tiles
NIB = N // 512   # 8 i-blocks
NTB = N // 512   # 8 token blocks (phase 1)
NKT = C // 128   # 6 contraction tiles
NF = 704         # features computed in phase 1 (q01,k01,v01,q22,k22,v2)


def _build():
    nc = bacc.Bacc("TRN2", target_bir_lowering=False, debug=False,
                   num_devices=NCORES)

    xT = nc.dram_tensor("xT", [C, N], F16, kind="ExternalInput").ap()
    w = nc.dram_tensor("w", [C, NF], F16, kind="ExternalInput").ap()
    bias = nc.dram_tensor("bias", [128, 6], F32, kind="ExternalInput").ap()
    pw = nc.dram_tensor("pw", [HPC * HD, C], F16, kind="ExternalInput").ap()
    y = nc.dram_tensor("y", [C, N], F16, kind="ExternalOutput").ap()

    xT_r = xT.rearrange("(kt p) (tb n) -> p tb kt n", p=128, n=512)
    w_r = w.rearrange("(kt p) f -> p kt f", p=128)

    with tile.TileContext(nc) as tc:
        with (
            tc.tile_pool(name="singles", bufs=1) as singles,
            tc.tile_pool(name="bigs", bufs=1) as bigs,
            tc.tile_pool(name="xin", bufs=3) as xin,
            tc.tile_pool(name="exp", bufs=6) as expool,
            tc.tile_pool(name="outs", bufs=2) as outs,
            tc.tile_pool(name="psum", bufs=2, space="PSUM") as psum,
            tc.tile_pool(name="dram", bufs=1, space="DRAM") as dram,
        ):
            # ---- constants / weights ----
            w_sb = singles.tile([128, NKT, NF], F16)
            nc.sync.dma_start(out=w_sb, in_=w_r)
            bias_sb = singles.tile([128, 6], F32)
            nc.sync.dma_start(out=bias_sb, in_=bias)
            pwa = singles.tile([128, C], F16)
            nc.sync.dma_start(out=pwa, in_=pw[0:128, :])
            pwb = singles.tile([64, C], F16)
            nc.sync.dma_start(out=pwb, in_=pw[128:192, :])
            ident = singles.tile([128, 128], F16)
            make_identity(nc, ident)

            # ---- phase-1 destinations (feature-major, fp16) ----
            q01 = bigs.tile([128, N], F16)
            k01 = bigs.tile([128, N], F16)
            v01T = bigs.tile([128, N], F16)
            q22 = bigs.tile([128, N], F16)
            k22 = bigs.tile([128, N], F16)
            v2T = bigs.tile([64, N], F16)
            dests = [q01, k01, v01T, q22, k22, v2T]
            msizes = [128, 128, 128, 128, 128, 64]
            moffs = [0, 128, 256, 384, 512, 640]
            vaug = [bigs.tile([128, NT, 65], F16, name=f"vaug{h}",
                              tag=f"vaug{h}")
                    for h in range(HPC)]
            for h in range(HPC):
                nc.vector.memset(vaug[h][:, :, 64:65], 1.0)

            # ---- phase 1: qkv projection (k/q first so attention overlaps;
            #      two m-tiles per x load) ----
            for mta, mtb in ((1, 0), (2, 4), (3, 5)):
                for tb in range(NTB):
                    x_t = xin.tile([128, NKT, 512], F16, bufs=3)
                    nc.sync.dma_start(out=x_t, in_=xT_r[:, tb, :, :])
                    for mt in (mta, mtb):
                        msz = msizes[mt]
                        ps = psum.tile([128, 512], F32, tag="ps512", bufs=2)
                        for kt in range(NKT):
                            nc.tensor.matmul(
                                ps[0:msz, :],
                                lhsT=w_sb[:, kt, moffs[mt]:moffs[mt] + msz],
                                rhs=x_t[:, kt, :],
                                start=(kt == 0), stop=(kt == NKT - 1),
                            )
                        nc.vector.tensor_scalar_add(
                            out=dests[mt][0:msz, tb * 512:(tb + 1) * 512],
                            in0=ps[0:msz, :],
                            scalar1=bias_sb[0:msz, mt:mt + 1],
                        )
                    if mta == 2:
                        # v01 slice ready: transpose its 4 j-tiles now
                        for jt in range(4 * tb, 4 * tb + 4):
                            tp = psum.tile([128, 128], F16, tag="ps512",
                                           bufs=2)
                            nc.tensor.transpose(
                                tp, v01T[:, jt * 128:(jt + 1) * 128], ident)
                            nc.vector.tensor_copy(out=vaug[0][:, jt, 0:64],
                                                  in_=tp[:, 0:64])
                            nc.vector.tensor_copy(out=vaug[1][:, jt, 0:64],
                                                  in_=tp[:, 64:128])
                    if mtb == 5:
                        for jt in range(4 * tb, 4 * tb + 4):
                            tp2 = psum.tile([128, 64], F16, tag="ps512",
                                            bufs=2)
                            nc.tensor.transpose(
                                tp2, v2T[:, jt * 128:(jt + 1) * 128],
                                ident[0:64, 0:64])
                            nc.vector.tensor_copy(out=vaug[2][:, jt, 0:64],
                                                  in_=tp2)

            # ---- attention ----
            o01 = bigs.tile([128, N], F16)
            o2 = bigs.tile([64, N], F16)
            rec_d = dram.tile([24, 512], F32)     # 1/sums bounce for p-bcast


            def emit_proj(ib, tags):
                isl = slice(ib * 512, (ib + 1) * 512)
                for mtp in range(0, 6, 2):
                    psys = []
                    for k in range(2):
                        mt = mtp + k
                        msl = slice(mt * 128, (mt + 1) * 128)
                        psy = psum.tile([128, 512], F32,
                                        tag=tags[k % len(tags)],
                                        bufs=2, name=f"psy{k}")
                        nc.tensor.matmul(psy, lhsT=pwa[:, msl],
                                         rhs=o01[:, isl],
                                         start=True, stop=False)
                        psys.append((psy, mt))
                    for psy, mt in psys:
                        msl = slice(mt * 128, (mt + 1) * 128)
                        nc.tensor.matmul(psy, lhsT=pwb[:, msl],
                                         rhs=o2[:, isl],
                                         start=False, stop=True)
                        ysb = outs.tile([128, 512], F16, tag="ysb", bufs=4)
                        nc.vector.tensor_copy(out=ysb, in_=psy)
                        nc.sync.dma_start(out=y[msl, isl], in_=ysb)

            def emit_pair_scores(ib, jt):
                isl2 = slice(ib * 512, (ib + 1) * 512)
                jsl = slice(jt * 128, (jt + 1) * 128)
                sc = psum.tile([128, 1024], F32, tag="sc", bufs=2, name="sc")
                nc.tensor.matmul(sc[:, 0:512], lhsT=k01[0:64, jsl],
                                 rhs=q01[0:64, isl2], start=True, stop=True)
                nc.tensor.matmul(sc[:, 512:1024], lhsT=k01[64:128, jsl],
                                 rhs=q01[64:128, isl2], start=True, stop=True)
                ex = expool.tile([128, 1024], F16, name="ex")
                nc.scalar.activation(ex, sc, EXP)
                return ex

            def emit_norm(ib, coll):
                isl = slice(ib * 512, (ib + 1) * 512)
                rec = outs.tile([3, 512], F32, tag="rec")
                nc.vector.reciprocal(out=rec, in_=coll)
                nc.sync.dma_start(out=rec_d[3 * ib:3 * ib + 3, :], in_=rec)
                for h in range(HPC):
                    rb = outs.tile([128, 512], F32, tag="rb")
                    src = rec_d[3 * ib + h:3 * ib + h + 1, :]
                    if h == 1:
                        rbv = rb[64:128, :]
                        dst = o01[64:128, isl]
                    elif h == 0:
                        rbv = rb[0:64, :]
                        dst = o01[0:64, isl]
                    else:
                        rbv = rb[0:64, :]
                        dst = o2[:, isl]
                    nc.sync.dma_start(out=rbv, in_=src.broadcast_to([64, 512]))
                    nc.vector.tensor_mul(dst, dst, rbv)

            prev = None
            for ib in range(NIB):
                isl = slice(ib * 512, (ib + 1) * 512)
                coll = outs.tile([3, 512], F32, tag="coll")

                # -- head pair (h0, h1): concurrent scores via row halves --
                pv0 = psum.tile([65, 512], F32, tag="pv", bufs=2)
                pv1 = psum.tile([65, 512], F32, tag="pv", bufs=2)
                for jt in range(NT):
                    ex = emit_pair_scores(ib, jt)
                    nc.tensor.matmul(pv0, lhsT=vaug[0][:, jt, :],
                                     rhs=ex[:, 0:512],
                                     start=(jt == 0), stop=(jt == NT - 1))
                    nc.tensor.matmul(pv1, lhsT=vaug[1][:, jt, :],
                                     rhs=ex[:, 512:1024],
                                     start=(jt == 0), stop=(jt == NT - 1))
                nc.vector.tensor_copy(out=o01[0:64, isl], in_=pv0[0:64, :])
                nc.vector.tensor_copy(out=o01[64:128, isl], in_=pv1[0:64, :])
                sst0 = outs.tile([128, 512], F32, tag="sstage")
                nc.vector.tensor_copy(out=sst0[64:65, :], in_=pv0[64:65, :])
                nc.sync.dma_start(out=coll[0:1, :], in_=sst0[64:65, :])
                sst1 = outs.tile([128, 512], F32, tag="sstage")
                nc.vector.tensor_copy(out=sst1[64:65, :], in_=pv1[64:65, :])
                nc.sync.dma_start(out=coll[1:2, :], in_=sst1[64:65, :])

                # -- head 2: even/odd j-tiles concurrent via row halves --
                pv2 = psum.tile([65, 512], F32, tag="pv", bufs=2)
                for jg in range(NT // 2):
                    jte, jto = 2 * jg, 2 * jg + 1
                    esl = slice(jte * 128, (jte + 1) * 128)
                    osl = slice(jto * 128, (jto + 1) * 128)
                    sc2 = psum.tile([128, 1024], F32, tag="sc", bufs=2)
                    nc.tensor.matmul(sc2[:, 0:512], lhsT=k22[0:64, esl],
                                     rhs=q22[0:64, isl], start=True, stop=True)
                    nc.tensor.matmul(sc2[:, 512:1024], lhsT=k22[64:128, osl],
                                     rhs=q22[64:128, isl], start=True, stop=True)
                    ex2 = expool.tile([128, 1024], F16)
                    nc.scalar.activation(ex2, sc2, EXP)
                    nc.tensor.matmul(pv2, lhsT=vaug[2][:, jte, :],
                                     rhs=ex2[:, 0:512],
                                     start=(jg == 0), stop=False)
                    nc.tensor.matmul(pv2, lhsT=vaug[2][:, jto, :],
                                     rhs=ex2[:, 512:1024],
                                     start=False, stop=(jg == NT // 2 - 1))
                nc.vector.tensor_copy(out=o2[:, isl], in_=pv2[0:64, :])
                sst2 = outs.tile([128, 512], F32, tag="sstage")
                nc.vector.tensor_copy(out=sst2[64:65, :], in_=pv2[64:65, :])
                nc.sync.dma_start(out=coll[2:3, :], in_=sst2[64:65, :])

                # -- normalize the previous i-block (stagger smooths FIFO) --
                if prev is not None:
                    emit_norm(prev[0], prev[1])
                prev = (ib, coll)
            emit_norm(prev[0], prev[1])

            for ib in range(NIB):
                emit_proj(ib, ("ps512", "pv", "sc"))

    nc.finalize()
    return nc


_NC_CACHE = None


def _get_nc():
    global _NC_CACHE
    if _NC_CACHE is None:
        _NC_CACHE = _build()
    return _NC_CACHE


def _prep_core_inputs(x, qkv_w, qkv_b, proj_w, core):
    """Build the per-core input dict (numpy, host-side)."""
    b, g = core // 4, core % 4
    h = [3 * g, 3 * g + 1, 3 * g + 2]

    xT = np.ascontiguousarray(
        x[b].reshape(N, C).T.astype(np.float16))          # (768, 4096)

    def wq(head):  # scaled q rows, (64, 768)
        return qkv_w[HD * head:HD * (head + 1), :] * SCALE

    def wk(head):
        return qkv_w[C + HD * head:C + HD * (head + 1), :]

    def wv(head):
        return qkv_w[2 * C + HD * head:2 * C + HD * (head + 1), :]

    def bq(head):
        return qkv_b[HD * head:HD * (head + 1)] * SCALE

    def bk(head):
        return qkv_b[C + HD * head:C + HD * (head + 1)]

    def bv(head):
        return qkv_b[2 * C + HD * head:2 * C + HD * (head + 1)]

    # feature columns: q01 | k01 | v01 | q22 | k22 | v2   (704 total)
    wcols = np.concatenate([
        wq(h[0]), wq(h[1]), wk(h[0]), wk(h[1]), wv(h[0]), wv(h[1]),
        wq(h[2]), wq(h[2]), wk(h[2]), wk(h[2]), wv(h[2]),
    ], axis=0)                                            # (704, 768)
    w = np.ascontiguousarray(wcols.T.astype(np.float16))  # (768, 704)

    bcols = np.concatenate([
        bq(h[0]), bq(h[1]), bk(h[0]), bk(h[1]), bv(h[0]), bv(h[1]),
        bq(h[2]), bq(h[2]), bk(h[2]), bk(h[2]), bv(h[2]),
        np.zeros(64, np.float32),
    ]).astype(np.float32)                                 # (768,)
    bias = np.ascontiguousarray(bcols.reshape(6, 128).T)  # (128, 6)

    ch = slice(HPC * HD * g, HPC * HD * (g + 1))
    pw = np.ascontiguousarray(proj_w[:, ch].T.astype(np.float16))  # (192, 768)

    return {"xT": xT, "w": w, "bias": bias, "pw": pw}


def kernel(x, qkv_w, qkv_b, proj_w, proj_b):
    x = np.asarray(x, np.float32)
    qkv_w = np.asarray(qkv_w, np.float32)
    qkv_b = np.asarray(qkv_b, np.float32)
    proj_w = np.asarray(proj_w, np.float32)
    proj_b = np.asarray(proj_b, np.float32)

    nc = _get_nc()
    in_maps = [_prep_core_inputs(x, qkv_w, qkv_b, proj_w, c)
               for c in range(NCORES)]
    res = run_bass_kernel_spmd(nc, in_maps, list(range(NCORES)))

    out = np.empty((B, N, C), np.float32)
    for b in range(B):
        acc = np.zeros((C, N), np.float32)
        for g in range(4):
            acc += res.results[b * 4 + g]["y"].astype(np.float32)
        out[b] = acc.T + proj_b[None, :]
    return out


if __name__ == "__main__":
    rng = np.random.default_rng(0)
    x = rng.standard_normal((B, 64, 64, C), np.float32)
    qkv_w = (rng.standard_normal((3 * C, C), np.float32) * 0.02)
    qkv_b = (rng.standard_normal(3 * C, np.float32) * 0.02)
    proj_w = (rng.standard_normal((C, C), np.float32) * 0.02)
    proj_b = (rng.standard_normal(C, np.float32) * 0.02)
    out = kernel(x=x, qkv_w=qkv_w, qkv_b=qkv_b, proj_w=proj_w, proj_b=proj_b)
    print("out", out.shape, out.dtype, float(np.abs(out).max()))
